# revision 19
# baseline (speedup 1.0000x reference)
"""Additive-attention (Bahdanau) kernel for Trainium2, 8 NeuronCores.

Computes attns[b, n, m] = sum_h v[h] * tanh(hq[b, h, n] + hk[b, h, m])
where hq = Wq @ q[b], hk = Wk @ k[b], returned flattened as (B, NQ*NK).

Strategy (data-parallel over batch, 4 batches per core):
  - hq/hk via fp32 PE matmuls (host-pretransposed W as lhsT), cast to fp16.
  - preact[h, (n,m)] = hk + hq[:, n] built per-query with DVE
    tensor_scalar_add (per-partition scalar operand -> 4x mode on fp16).
  - tanh on ScalarE in big (128, 8192) instructions (the bottleneck engine:
    ~16.8M tanh elems/core at 128/cycle @ 1.2 GHz ~= 110us).
  - v-contraction over h on PE: v_half (128,1) stationary, tanh slab rhs
    N=512 per matmul, 2 h-halves accumulated in PSUM; 4 query-pairs share
    one PSUM bank via col-tiling (tile_position) at partitions 0/32/64/96.
  - PSUM->SBUF on DVE, one strided DMA per 4-pair group to HBM.
"""

import sys

sys.path.insert(0, "/opt/trn_rl_repo")

from contextlib import ExitStack

import numpy as np

import concourse.bacc as bacc
import concourse.bass as bass
import concourse.mybir as mybir
import concourse.tile as tile
from concourse.bass_utils import run_bass_kernel_spmd

B, HID, QH, KH, NQ, NK = 32, 256, 256, 256, 64, 256
NCORES = 8
BPC = B // NCORES  # batches per core
NCHUNK = 2  # query chunks per batch
QPC = NQ // NCHUNK  # queries per chunk (32)
PAIRS = QPC // 2  # query pairs per chunk (16)
GROUPS = PAIRS // 4  # groups of 4 pairs per chunk (4)

f32 = mybir.dt.float32
f16 = mybir.dt.float16

_NC_CACHE = {}


def build_nc():
    nc = bacc.Bacc("TRN2", target_bir_lowering=False, debug=False)

    q_d = nc.dram_tensor("q", [BPC, 2, 128, NQ], f32, kind="ExternalInput")
    k_d = nc.dram_tensor("k", [BPC, 2, 128, NK], f32, kind="ExternalInput")
    wqt_d = nc.dram_tensor("wqt", [2, 128, HID], f32, kind="ExternalInput")
    wkt_d = nc.dram_tensor("wkt", [2, 128, HID], f32, kind="ExternalInput")
    vh_d = nc.dram_tensor("vh", [128, 64], f16, kind="ExternalInput")
    out_d = nc.dram_tensor("out", [BPC, 2 * GROUPS, 4, 512], f32, kind="ExternalOutput")

    with tile.TileContext(nc) as tc, ExitStack() as ctx:
        wpool = ctx.enter_context(tc.tile_pool(name="wpool", bufs=1))
        iopool = ctx.enter_context(tc.tile_pool(name="iopool", bufs=3))
        hpool = ctx.enter_context(tc.tile_pool(name="hpool", bufs=3))
        prepool = ctx.enter_context(tc.tile_pool(name="prepool", bufs=3))
        tanhpool = ctx.enter_context(tc.tile_pool(name="tanhpool", bufs=4))
        obpool = ctx.enter_context(tc.tile_pool(name="obpool", bufs=6))
        psA = ctx.enter_context(tc.tile_pool(name="psA", bufs=2, space="PSUM"))
        psO = ctx.enter_context(tc.tile_pool(name="psO", bufs=3, space="PSUM"))

        # Preload the tanh ACT table at t=0 (overlaps with input DMAs).
        warm = wpool.tile([128, 2], f16, name="warm", tag="warm")
        nc.vector.memset(warm[:, 0:1], 0.0)
        nc.scalar.activation(
            warm[:, 1:2], warm[:, 0:1], mybir.ActivationFunctionType.Tanh
        )

        def load_qk(b, eng=None):
            eng = eng or nc.gpsimd
            q_sb = iopool.tile([128, 2 * NQ], f32, name=f"q_sb{b}", tag="qsb")
            k_sb = iopool.tile([128, 2 * NK], f32, name=f"k_sb{b}", tag="ksb")
            for kb in range(2):
                eng.dma_start(q_sb[:, bass.ts(kb, NQ)], q_d[b, kb])
                eng.dma_start(k_sb[:, bass.ts(kb, NK)], k_d[b, kb])
            return q_sb, k_sb

        q0_sb = iopool.tile([128, 2 * NQ], f32, name="q_sb0", tag="qsb")
        k0_sb = iopool.tile([128, 2 * NK], f32, name="k_sb0", tag="ksb")
        wq_sb = []
        wk_sb = []
        for kb in range(2):
            wq_t = wpool.tile([128, HID], f32, name=f"wq_sb{kb}", tag=f"wq{kb}")
            wq_sb.append(wq_t)
            wk_t = wpool.tile([128, HID], f32, name=f"wk_sb{kb}", tag=f"wk{kb}")
            wk_sb.append(wk_t)
        vh_sb = wpool.tile([128, 64], f16, name="vh_sb", tag="vh")
        # Split critical startup DMA issue across the two HWDGE engines:
        # sync takes the hq inputs, scalar takes the hk inputs.
        for kb in range(2):
            nc.sync.dma_start(q0_sb[:, bass.ts(kb, NQ)], q_d[0, kb])
            nc.sync.dma_start(wq_sb[kb][:], wqt_d[kb])
        for kb in range(2):
            nc.scalar.dma_start(k0_sb[:, bass.ts(kb, NK)], k_d[0, kb])
            nc.scalar.dma_start(wk_sb[kb][:], wkt_d[kb])
        nc.scalar.dma_start(vh_sb[:], vh_d[:])
        qk = {0: (q0_sb, k0_sb)}
        hqhk = {}
        deferred = []
        done_tiles = []

        def make_hqhk(b):
            cast = nc.scalar.copy if b == 0 else nc.vector.tensor_copy
            q_sb, k_sb = qk.pop(b)
            hq32 = hpool.tile([128, 2 * NQ], f32, name=f"hq32_{b}", tag="hq32")
            hk16 = hpool.tile([128, 2 * NK], f16, name=f"hk16_{b}", tag="hk16")
            for j in range(2):
                ps_hq = psA.tile([128, NQ], f32, name=f"ps_hq{b}_{j}", tag="psA")
                for kb in range(2):
                    nc.tensor.matmul(
                        ps_hq[:],
                        wq_sb[kb][:, bass.ts(j, 128)],
                        q_sb[:, bass.ts(kb, NQ)],
                        start=(kb == 0),
                        stop=(kb == 1),
                    )
                cast(hq32[:, bass.ts(j, NQ)], ps_hq[:])
                ps_hk = psA.tile([128, NK], f32, name=f"ps_hk{b}_{j}", tag="psA")
                for kb in range(2):
                    nc.tensor.matmul(
                        ps_hk[:],
                        wk_sb[kb][:, bass.ts(j, 128)],
                        k_sb[:, bass.ts(kb, NK)],
                        start=(kb == 0),
                        stop=(kb == 1),
                    )
                cast(hk16[:, bass.ts(j, NK)], ps_hk[:])
            hqhk[b] = (hq32, hk16)

        make_hqhk(0)
        qk[1] = load_qk(1)
        make_hqhk(1)
        qk[2] = load_qk(2)

        # Work units: (batch, qlo, nq). Fine-grained at the start so ACT
        # ramps early, 16-query pieces at the end for a short drain; full
        # 32-query chunks in steady state.
        units = []
        for b in range(BPC):
            if b == 0:
                units += [(0, 0, 8), (0, 8, 8), (0, 16, 16), (0, 32, 32)]
            elif b == BPC - 1:
                units += [(b, 0, 32), (b, 32, 16), (b, 48, 16)]
            else:
                units += [(b, 0, 32), (b, 32, 32)]

        deferred = []
        seen_chunks = 0
        for ui, (b, qlo, nq) in enumerate(units):
            hq32, hk16 = hqhk[b]
            if qlo == 0:
                if b + 2 < BPC:
                    make_hqhk(b + 2)
                if b + 3 < BPC:
                    qk[b + 3] = load_qk(b + 3)

            th = []
            for j in range(2):
                pre = prepool.tile(
                    [128, nq * NK], f16, name=f"pre{b}_{qlo}_{j}", tag="pre"
                )
                for nn in range(nq):
                    n = qlo + nn
                    nc.vector.tensor_scalar_add(
                        pre[:, bass.ts(nn, NK)],
                        hk16[:, bass.ts(j, NK)],
                        hq32[:, j * NQ + n : j * NQ + n + 1],
                    )
                t_ = tanhpool.tile(
                    [128, nq * NK], f16, name=f"tanh{b}_{qlo}_{j}", tag="tanh"
                )
                nc.scalar.activation(t_[:], pre[:], mybir.ActivationFunctionType.Tanh)
                th.append(t_)

            tails = []
            ngroups = nq // 8
            g = 0
            while g < ngroups:
                w = 2 if ngroups - g >= 2 else 1  # banks per psum tile
                ps = psO.tile(
                    [128, 512 * w], f32, name=f"ps{b}_{qlo}_{g}", tag="psO"
                )
                for gg in range(w):
                    for j in range(2):
                        for r in range(4):
                            p = 4 * (g + gg) + r
                            nc.tensor.matmul(
                                ps[32 * r : 32 * r + 32, bass.ts(gg, 512)],
                                vh_sb[:, bass.ts(j, 32)],
                                th[j][:, bass.ts(p, 512)],
                                start=(j == 0),
                                stop=(j == 1),
                                tile_position=(0, 32 * r),
                                skip_group_check=True,
                            )
                tails.append((b, qlo // 8 + g, w, ps))
                g += w

            for bb, gg, w, pss in deferred:
                ob = obpool.tile(
                    [128, 512 * w], f32, name=f"ob{bb}_{gg}", tag="ob"
                )
                if gg % 4 == 2:
                    nc.scalar.copy(ob[:], pss[:])
                else:
                    nc.vector.tensor_copy(ob[:], pss[:])
                dst = out_d[bb, gg : gg + w].rearrange("g r c -> r g c")
                srcap = ob[0:128:32, :].rearrange("p (g c) -> p g c", g=w)
                nc.sync.dma_start(dst, srcap)
            deferred = tails

        for bb, gg, w, pss in deferred:
            ob = obpool.tile([128, 512 * w], f32, name=f"ob{bb}_{gg}", tag="ob")
            nc.vector.tensor_copy(ob[:], pss[:])
            dst = out_d[bb, gg : gg + w].rearrange("g r c -> r g c")
            srcap = ob[0:128:32, :].rearrange("p (g c) -> p g c", g=w)
            nc.sync.dma_start(dst, srcap)

    nc.compile()
    return nc


def get_nc():
    if "nc" not in _NC_CACHE:
        _NC_CACHE["nc"] = build_nc()
    return _NC_CACHE["nc"]


def make_in_maps(att_query, att_key, v, W):
    att_query = np.ascontiguousarray(np.asarray(att_query, dtype=np.float32))
    att_key = np.ascontiguousarray(np.asarray(att_key, dtype=np.float32))
    v = np.asarray(v, dtype=np.float32)
    W = np.asarray(W, dtype=np.float32)

    q_all = att_query.reshape(NCORES, BPC, 2, 128, NQ)
    k_all = att_key.reshape(NCORES, BPC, 2, 128, NK)
    wqt = np.ascontiguousarray(W[:, :QH].T).reshape(2, 128, HID)
    wkt = np.ascontiguousarray(W[:, QH:].T).reshape(2, 128, HID)
    vh = np.ascontiguousarray(np.repeat(v.astype(np.float16).reshape(2, 128).T, 32, axis=1))

    return [
        {
            "q": np.ascontiguousarray(q_all[c]),
            "k": np.ascontiguousarray(k_all[c]),
            "wqt": wqt,
            "wkt": wkt,
            "vh": vh,
        }
        for c in range(NCORES)
    ]


def _ensure_ntff_hook():
    """Register the axon NTFF profile hook (image's antenv lacks axon_hooks)."""
    import types

    try:
        import antenv.axon_hooks  # noqa: F401
    except ImportError:
        import antenv

        mod = types.ModuleType("antenv.axon_hooks")
        _hook = [None]
        mod.set_axon_ntff_profile_hook = lambda h: _hook.__setitem__(0, h)
        mod.get_axon_ntff_profile_hook = lambda: _hook[0]
        sys.modules["antenv.axon_hooks"] = mod
        antenv.axon_hooks = mod
    from antenv.axon_hooks import (
        get_axon_ntff_profile_hook,
        set_axon_ntff_profile_hook,
    )

    if get_axon_ntff_profile_hook() is None:
        from trn_agent_boot.trn_boot import _ntff_profile_via_ctypes

        set_axon_ntff_profile_hook(_ntff_profile_via_ctypes("/opt/axon/libaxon_pjrt.so"))


def run(att_query, att_key, v, W, trace=False, **kwargs):
    nc = get_nc()
    if trace:
        _ensure_ntff_hook()
    in_maps = make_in_maps(att_query, att_key, v, W)
    res = run_bass_kernel_spmd(
        nc, in_maps, core_ids=list(range(NCORES)), trace=trace, **kwargs
    )
    outs = [np.asarray(res.results[c]["out"]).reshape(BPC, NQ * NK) for c in range(NCORES)]
    return np.concatenate(outs, axis=0), res


def kernel(att_query, att_key, v, W):
    out, _ = run(att_query, att_key, v, W)
    return out


# revision 20
# speedup vs baseline: 1.0030x; 1.0030x over previous
"""Additive-attention (Bahdanau) kernel for Trainium2, 8 NeuronCores.

Computes attns[b, n, m] = sum_h v[h] * tanh(hq[b, h, n] + hk[b, h, m])
where hq = Wq @ q[b], hk = Wk @ k[b], returned flattened as (B, NQ*NK).

Strategy (data-parallel over batch, 4 batches per core):
  - hq/hk via fp32 PE matmuls (host-pretransposed W as lhsT), cast to fp16.
  - preact[h, (n,m)] = hk + hq[:, n] built per-query with DVE
    tensor_scalar_add (per-partition scalar operand -> 4x mode on fp16).
  - tanh on ScalarE in big (128, 8192) instructions (the bottleneck engine:
    ~16.8M tanh elems/core at 128/cycle @ 1.2 GHz ~= 110us).
  - v-contraction over h on PE: v_half (128,1) stationary, tanh slab rhs
    N=512 per matmul, 2 h-halves accumulated in PSUM; 4 query-pairs share
    one PSUM bank via col-tiling (tile_position) at partitions 0/32/64/96.
  - PSUM->SBUF on DVE, one strided DMA per 4-pair group to HBM.
"""

import sys

sys.path.insert(0, "/opt/trn_rl_repo")

from contextlib import ExitStack

import numpy as np

import concourse.bacc as bacc
import concourse.bass as bass
import concourse.mybir as mybir
import concourse.tile as tile
from concourse.bass_utils import run_bass_kernel_spmd

B, HID, QH, KH, NQ, NK = 32, 256, 256, 256, 64, 256
NCORES = 8
BPC = B // NCORES  # batches per core
NCHUNK = 2  # query chunks per batch
QPC = NQ // NCHUNK  # queries per chunk (32)
PAIRS = QPC // 2  # query pairs per chunk (16)
GROUPS = PAIRS // 4  # groups of 4 pairs per chunk (4)

f32 = mybir.dt.float32
f16 = mybir.dt.float16

_NC_CACHE = {}


def build_nc():
    nc = bacc.Bacc("TRN2", target_bir_lowering=False, debug=False)

    q_d = nc.dram_tensor("q", [BPC, 2, 128, NQ], f32, kind="ExternalInput")
    k_d = nc.dram_tensor("k", [BPC, 2, 128, NK], f32, kind="ExternalInput")
    wqt_d = nc.dram_tensor("wqt", [2, 128, HID], f32, kind="ExternalInput")
    wkt_d = nc.dram_tensor("wkt", [2, 128, HID], f32, kind="ExternalInput")
    vh_d = nc.dram_tensor("vh", [128, 64], f16, kind="ExternalInput")
    out_d = nc.dram_tensor("out", [BPC, 2 * GROUPS, 4, 512], f32, kind="ExternalOutput")

    with tile.TileContext(nc) as tc, ExitStack() as ctx:
        wpool = ctx.enter_context(tc.tile_pool(name="wpool", bufs=1))
        iopool = ctx.enter_context(tc.tile_pool(name="iopool", bufs=3))
        hpool = ctx.enter_context(tc.tile_pool(name="hpool", bufs=3))
        prepool = ctx.enter_context(tc.tile_pool(name="prepool", bufs=3))
        tanhpool = ctx.enter_context(tc.tile_pool(name="tanhpool", bufs=4))
        obpool = ctx.enter_context(tc.tile_pool(name="obpool", bufs=6))
        psA = ctx.enter_context(tc.tile_pool(name="psA", bufs=2, space="PSUM"))
        psO = ctx.enter_context(tc.tile_pool(name="psO", bufs=3, space="PSUM"))

        # Preload the tanh ACT table at t=0 (overlaps with input DMAs).
        warm = wpool.tile([128, 2], f16, name="warm", tag="warm")
        nc.vector.memset(warm[:, 0:1], 0.0)
        nc.scalar.activation(
            warm[:, 1:2], warm[:, 0:1], mybir.ActivationFunctionType.Tanh
        )

        def load_qk(b, eng=None):
            eng = eng or nc.gpsimd
            q_sb = iopool.tile([128, 2 * NQ], f32, name=f"q_sb{b}", tag="qsb")
            k_sb = iopool.tile([128, 2 * NK], f32, name=f"k_sb{b}", tag="ksb")
            for kb in range(2):
                eng.dma_start(q_sb[:, bass.ts(kb, NQ)], q_d[b, kb])
                eng.dma_start(k_sb[:, bass.ts(kb, NK)], k_d[b, kb])
            return q_sb, k_sb

        q0_sb = iopool.tile([128, 2 * NQ], f32, name="q_sb0", tag="qsb")
        k0_sb = iopool.tile([128, 2 * NK], f32, name="k_sb0", tag="ksb")
        wq_sb = []
        wk_sb = []
        for kb in range(2):
            wq_t = wpool.tile([128, HID], f32, name=f"wq_sb{kb}", tag=f"wq{kb}")
            wq_sb.append(wq_t)
            wk_t = wpool.tile([128, HID], f32, name=f"wk_sb{kb}", tag=f"wk{kb}")
            wk_sb.append(wk_t)
        vh_sb = wpool.tile([128, 64], f16, name="vh_sb", tag="vh")
        # Split critical startup DMA issue across the two HWDGE engines:
        # sync takes the hq inputs, scalar takes the hk inputs.
        for kb in range(2):
            nc.sync.dma_start(q0_sb[:, bass.ts(kb, NQ)], q_d[0, kb])
            nc.sync.dma_start(wq_sb[kb][:], wqt_d[kb])
        for kb in range(2):
            nc.scalar.dma_start(k0_sb[:, bass.ts(kb, NK)], k_d[0, kb])
            nc.scalar.dma_start(wk_sb[kb][:], wkt_d[kb])
        nc.scalar.dma_start(vh_sb[:], vh_d[:])
        qk = {0: (q0_sb, k0_sb)}
        hqhk = {}
        deferred = []
        done_tiles = []

        def make_hqhk(b):
            cast = nc.scalar.copy if b == 0 else nc.vector.tensor_copy
            q_sb, k_sb = qk.pop(b)
            hq32 = hpool.tile([128, 2 * NQ], f32, name=f"hq32_{b}", tag="hq32")
            hk16 = hpool.tile([128, 2 * NK], f16, name=f"hk16_{b}", tag="hk16")
            for j in range(2):
                ps_hq = psA.tile([128, NQ], f32, name=f"ps_hq{b}_{j}", tag="psA")
                for kb in range(2):
                    nc.tensor.matmul(
                        ps_hq[:],
                        wq_sb[kb][:, bass.ts(j, 128)],
                        q_sb[:, bass.ts(kb, NQ)],
                        start=(kb == 0),
                        stop=(kb == 1),
                    )
                cast(hq32[:, bass.ts(j, NQ)], ps_hq[:])
                ps_hk = psA.tile([128, NK], f32, name=f"ps_hk{b}_{j}", tag="psA")
                for kb in range(2):
                    nc.tensor.matmul(
                        ps_hk[:],
                        wk_sb[kb][:, bass.ts(j, 128)],
                        k_sb[:, bass.ts(kb, NK)],
                        start=(kb == 0),
                        stop=(kb == 1),
                    )
                cast(hk16[:, bass.ts(j, NK)], ps_hk[:])
            hqhk[b] = (hq32, hk16)

        make_hqhk(0)
        qk[1] = load_qk(1)
        make_hqhk(1)
        qk[2] = load_qk(2)

        # Work units: (batch, qlo, nq). Fine-grained at the start so ACT
        # ramps early, 16-query pieces at the end for a short drain; full
        # 32-query chunks in steady state.
        units = []
        for b in range(BPC):
            if b == 0:
                units += [(0, 0, 8), (0, 8, 8), (0, 16, 16), (0, 32, 32)]
            elif b == BPC - 1:
                units += [(b, 0, 32), (b, 32, 16), (b, 48, 16)]
            else:
                units += [(b, 0, 32), (b, 32, 32)]

        deferred = []
        seen_chunks = 0
        for ui, (b, qlo, nq) in enumerate(units):
            hq32, hk16 = hqhk[b]
            if qlo == 0:
                if b + 2 < BPC:
                    make_hqhk(b + 2)
                if b + 3 < BPC:
                    qk[b + 3] = load_qk(b + 3)

            th = []
            for j in range(2):
                pre = prepool.tile(
                    [128, nq * NK], f16, name=f"pre{b}_{qlo}_{j}", tag="pre"
                )
                for nn in range(nq):
                    n = qlo + nn
                    nc.vector.tensor_scalar_add(
                        pre[:, bass.ts(nn, NK)],
                        hk16[:, bass.ts(j, NK)],
                        hq32[:, j * NQ + n : j * NQ + n + 1],
                    )
                t_ = tanhpool.tile(
                    [128, nq * NK], f16, name=f"tanh{b}_{qlo}_{j}", tag="tanh"
                )
                nc.scalar.activation(t_[:], pre[:], mybir.ActivationFunctionType.Tanh)
                th.append(t_)
                if j == 0:
                    for bb, gg, w, pss in deferred:
                        ob = obpool.tile(
                            [128, 512 * w], f32, name=f"ob{bb}_{gg}", tag="ob"
                        )
                        if gg == 6:
                            nc.scalar.copy(ob[:], pss[:])
                        else:
                            nc.vector.tensor_copy(ob[:], pss[:])
                        dst = out_d[bb, gg : gg + w].rearrange("g r c -> r g c")
                        srcap = ob[0:128:32, :].rearrange("p (g c) -> p g c", g=w)
                        nc.sync.dma_start(dst, srcap)
                    deferred = []

            tails = []
            ngroups = nq // 8
            g = 0
            while g < ngroups:
                w = 2 if ngroups - g >= 2 else 1  # banks per psum tile
                ps = psO.tile(
                    [128, 512 * w], f32, name=f"ps{b}_{qlo}_{g}", tag="psO"
                )
                for gg in range(w):
                    for j in range(2):
                        for r in range(4):
                            p = 4 * (g + gg) + r
                            nc.tensor.matmul(
                                ps[32 * r : 32 * r + 32, bass.ts(gg, 512)],
                                vh_sb[:, bass.ts(j, 32)],
                                th[j][:, bass.ts(p, 512)],
                                start=(j == 0),
                                stop=(j == 1),
                                tile_position=(0, 32 * r),
                                skip_group_check=True,
                            )
                tails.append((b, qlo // 8 + g, w, ps))
                g += w

            for bb, gg, w, pss in deferred:
                ob = obpool.tile(
                    [128, 512 * w], f32, name=f"ob{bb}_{gg}", tag="ob"
                )
                if gg == 6:
                    nc.scalar.copy(ob[:], pss[:])
                else:
                    nc.vector.tensor_copy(ob[:], pss[:])
                dst = out_d[bb, gg : gg + w].rearrange("g r c -> r g c")
                srcap = ob[0:128:32, :].rearrange("p (g c) -> p g c", g=w)
                nc.sync.dma_start(dst, srcap)
            deferred = tails

        for bb, gg, w, pss in deferred:
            ob = obpool.tile([128, 512 * w], f32, name=f"ob{bb}_{gg}", tag="ob")
            nc.vector.tensor_copy(ob[:], pss[:])
            dst = out_d[bb, gg : gg + w].rearrange("g r c -> r g c")
            srcap = ob[0:128:32, :].rearrange("p (g c) -> p g c", g=w)
            nc.sync.dma_start(dst, srcap)

    nc.compile()
    return nc


def get_nc():
    if "nc" not in _NC_CACHE:
        _NC_CACHE["nc"] = build_nc()
    return _NC_CACHE["nc"]


def make_in_maps(att_query, att_key, v, W):
    att_query = np.ascontiguousarray(np.asarray(att_query, dtype=np.float32))
    att_key = np.ascontiguousarray(np.asarray(att_key, dtype=np.float32))
    v = np.asarray(v, dtype=np.float32)
    W = np.asarray(W, dtype=np.float32)

    q_all = att_query.reshape(NCORES, BPC, 2, 128, NQ)
    k_all = att_key.reshape(NCORES, BPC, 2, 128, NK)
    wqt = np.ascontiguousarray(W[:, :QH].T).reshape(2, 128, HID)
    wkt = np.ascontiguousarray(W[:, QH:].T).reshape(2, 128, HID)
    vh = np.ascontiguousarray(np.repeat(v.astype(np.float16).reshape(2, 128).T, 32, axis=1))

    return [
        {
            "q": np.ascontiguousarray(q_all[c]),
            "k": np.ascontiguousarray(k_all[c]),
            "wqt": wqt,
            "wkt": wkt,
            "vh": vh,
        }
        for c in range(NCORES)
    ]


def _ensure_ntff_hook():
    """Register the axon NTFF profile hook (image's antenv lacks axon_hooks)."""
    import types

    try:
        import antenv.axon_hooks  # noqa: F401
    except ImportError:
        import antenv

        mod = types.ModuleType("antenv.axon_hooks")
        _hook = [None]
        mod.set_axon_ntff_profile_hook = lambda h: _hook.__setitem__(0, h)
        mod.get_axon_ntff_profile_hook = lambda: _hook[0]
        sys.modules["antenv.axon_hooks"] = mod
        antenv.axon_hooks = mod
    from antenv.axon_hooks import (
        get_axon_ntff_profile_hook,
        set_axon_ntff_profile_hook,
    )

    if get_axon_ntff_profile_hook() is None:
        from trn_agent_boot.trn_boot import _ntff_profile_via_ctypes

        set_axon_ntff_profile_hook(_ntff_profile_via_ctypes("/opt/axon/libaxon_pjrt.so"))


def run(att_query, att_key, v, W, trace=False, **kwargs):
    nc = get_nc()
    if trace:
        _ensure_ntff_hook()
    in_maps = make_in_maps(att_query, att_key, v, W)
    res = run_bass_kernel_spmd(
        nc, in_maps, core_ids=list(range(NCORES)), trace=trace, **kwargs
    )
    outs = [np.asarray(res.results[c]["out"]).reshape(BPC, NQ * NK) for c in range(NCORES)]
    return np.concatenate(outs, axis=0), res


def kernel(att_query, att_key, v, W):
    out, _ = run(att_query, att_key, v, W)
    return out


# revision 21
# speedup vs baseline: 1.0095x; 1.0065x over previous
"""Additive-attention (Bahdanau) kernel for Trainium2, 8 NeuronCores.

Computes attns[b, n, m] = sum_h v[h] * tanh(hq[b, h, n] + hk[b, h, m])
where hq = Wq @ q[b], hk = Wk @ k[b], returned flattened as (B, NQ*NK).

Strategy (data-parallel over batch, 4 batches per core):
  - hq/hk via fp32 PE matmuls (host-pretransposed W as lhsT), cast to fp16.
  - preact[h, (n,m)] = hk + hq[:, n] built per-query with DVE
    tensor_scalar_add (per-partition scalar operand -> 4x mode on fp16).
  - tanh on ScalarE in big (128, 8192) instructions (the bottleneck engine:
    ~16.8M tanh elems/core at 128/cycle @ 1.2 GHz ~= 110us).
  - v-contraction over h on PE: v_half (128,1) stationary, tanh slab rhs
    N=512 per matmul, 2 h-halves accumulated in PSUM; 4 query-pairs share
    one PSUM bank via col-tiling (tile_position) at partitions 0/32/64/96.
  - PSUM->SBUF on DVE, one strided DMA per 4-pair group to HBM.
"""

import sys

sys.path.insert(0, "/opt/trn_rl_repo")

from contextlib import ExitStack

import numpy as np

import concourse.bacc as bacc
import concourse.bass as bass
import concourse.mybir as mybir
import concourse.tile as tile
from concourse.bass_utils import run_bass_kernel_spmd

B, HID, QH, KH, NQ, NK = 32, 256, 256, 256, 64, 256
NCORES = 8
BPC = B // NCORES  # batches per core
NCHUNK = 2  # query chunks per batch
QPC = NQ // NCHUNK  # queries per chunk (32)
PAIRS = QPC // 2  # query pairs per chunk (16)
GROUPS = PAIRS // 4  # groups of 4 pairs per chunk (4)

f32 = mybir.dt.float32
f16 = mybir.dt.float16

_NC_CACHE = {}


def build_nc():
    nc = bacc.Bacc("TRN2", target_bir_lowering=False, debug=False)

    q_d = nc.dram_tensor("q", [BPC, 2, 128, NQ], f32, kind="ExternalInput")
    k_d = nc.dram_tensor("k", [BPC, 2, 128, NK], f32, kind="ExternalInput")
    wqt_d = nc.dram_tensor("wqt", [2, 128, HID], f32, kind="ExternalInput")
    wkt_d = nc.dram_tensor("wkt", [2, 128, HID], f32, kind="ExternalInput")
    vh_d = nc.dram_tensor("vh", [128, 64], f16, kind="ExternalInput")
    out_d = nc.dram_tensor("out", [BPC, 2 * GROUPS, 4, 512], f32, kind="ExternalOutput")

    with tile.TileContext(nc) as tc, ExitStack() as ctx:
        wpool = ctx.enter_context(tc.tile_pool(name="wpool", bufs=1))
        iopool = ctx.enter_context(tc.tile_pool(name="iopool", bufs=3))
        hpool = ctx.enter_context(tc.tile_pool(name="hpool", bufs=3))
        prepool = ctx.enter_context(tc.tile_pool(name="prepool", bufs=3))
        tanhpool = ctx.enter_context(tc.tile_pool(name="tanhpool", bufs=4))
        obpool = ctx.enter_context(tc.tile_pool(name="obpool", bufs=6))
        psA = ctx.enter_context(tc.tile_pool(name="psA", bufs=2, space="PSUM"))
        psO = ctx.enter_context(tc.tile_pool(name="psO", bufs=3, space="PSUM"))

        # Preload the tanh ACT table at t=0 (overlaps with input DMAs).
        warm = wpool.tile([128, 2], f16, name="warm", tag="warm")
        nc.vector.memset(warm[:, 0:1], 0.0)
        nc.scalar.activation(
            warm[:, 1:2], warm[:, 0:1], mybir.ActivationFunctionType.Tanh
        )

        def load_qk(b, eng=None):
            eng = eng or nc.gpsimd
            q_sb = iopool.tile([128, 2 * NQ], f32, name=f"q_sb{b}", tag="qsb")
            k_sb = iopool.tile([128, 2 * NK], f32, name=f"k_sb{b}", tag="ksb")
            eng.dma_start(
                q_sb[:].rearrange("p (kb n) -> p kb n", kb=2),
                q_d[b].rearrange("kb p n -> p kb n"),
            )
            eng.dma_start(
                k_sb[:].rearrange("p (kb n) -> p kb n", kb=2),
                k_d[b].rearrange("kb p n -> p kb n"),
            )
            return q_sb, k_sb

        q0_sb = iopool.tile([128, 2 * NQ], f32, name="q_sb0", tag="qsb")
        k0_sb = iopool.tile([128, 2 * NK], f32, name="k_sb0", tag="ksb")
        wq_sb = []
        wk_sb = []
        for kb in range(2):
            wq_t = wpool.tile([128, HID], f32, name=f"wq_sb{kb}", tag=f"wq{kb}")
            wq_sb.append(wq_t)
            wk_t = wpool.tile([128, HID], f32, name=f"wk_sb{kb}", tag=f"wk{kb}")
            wk_sb.append(wk_t)
        vh_sb = wpool.tile([128, 64], f16, name="vh_sb", tag="vh")
        # Split critical startup DMA issue across the two HWDGE engines:
        # sync takes the hq inputs, scalar takes the hk inputs. One DMA per
        # tensor (3D AP over the kb halves) to minimize serialized issues.
        nc.sync.dma_start(
            q0_sb[:].rearrange("p (kb n) -> p kb n", kb=2),
            q_d[0].rearrange("kb p n -> p kb n"),
        )
        nc.sync.dma_start(wq_sb[0][:], wqt_d[0])
        nc.sync.dma_start(wq_sb[1][:], wqt_d[1])
        nc.scalar.dma_start(
            k0_sb[:].rearrange("p (kb n) -> p kb n", kb=2),
            k_d[0].rearrange("kb p n -> p kb n"),
        )
        nc.scalar.dma_start(wk_sb[0][:], wkt_d[0])
        nc.scalar.dma_start(wk_sb[1][:], wkt_d[1])
        nc.scalar.dma_start(vh_sb[:], vh_d[:])
        qk = {0: (q0_sb, k0_sb)}
        hqhk = {}
        deferred = []
        done_tiles = []

        def make_hqhk(b):
            cast = nc.scalar.copy if b == 0 else nc.vector.tensor_copy
            q_sb, k_sb = qk.pop(b)
            hq32 = hpool.tile([128, 2 * NQ], f32, name=f"hq32_{b}", tag="hq32")
            hk16 = hpool.tile([128, 2 * NK], f16, name=f"hk16_{b}", tag="hk16")
            for j in range(2):
                ps_hq = psA.tile([128, NQ], f32, name=f"ps_hq{b}_{j}", tag="psA")
                for kb in range(2):
                    nc.tensor.matmul(
                        ps_hq[:],
                        wq_sb[kb][:, bass.ts(j, 128)],
                        q_sb[:, bass.ts(kb, NQ)],
                        start=(kb == 0),
                        stop=(kb == 1),
                    )
                cast(hq32[:, bass.ts(j, NQ)], ps_hq[:])
                ps_hk = psA.tile([128, NK], f32, name=f"ps_hk{b}_{j}", tag="psA")
                for kb in range(2):
                    nc.tensor.matmul(
                        ps_hk[:],
                        wk_sb[kb][:, bass.ts(j, 128)],
                        k_sb[:, bass.ts(kb, NK)],
                        start=(kb == 0),
                        stop=(kb == 1),
                    )
                cast(hk16[:, bass.ts(j, NK)], ps_hk[:])
            hqhk[b] = (hq32, hk16)

        make_hqhk(0)
        qk[1] = load_qk(1)
        make_hqhk(1)
        qk[2] = load_qk(2)

        # Work units: (batch, qlo, nq). Fine-grained at the start so ACT
        # ramps early, 16-query pieces at the end for a short drain; full
        # 32-query chunks in steady state.
        units = []
        for b in range(BPC):
            if b == 0:
                units += [(0, 0, 8), (0, 8, 8), (0, 16, 16), (0, 32, 32)]
            elif b == BPC - 1:
                units += [(b, 0, 32), (b, 32, 16), (b, 48, 16)]
            else:
                units += [(b, 0, 32), (b, 32, 32)]

        deferred = []
        seen_chunks = 0
        for ui, (b, qlo, nq) in enumerate(units):
            hq32, hk16 = hqhk[b]
            if qlo == 0:
                if b + 2 < BPC:
                    make_hqhk(b + 2)
                if b + 3 < BPC:
                    qk[b + 3] = load_qk(b + 3)

            th = []
            for j in range(2):
                pre = prepool.tile(
                    [128, nq * NK], f16, name=f"pre{b}_{qlo}_{j}", tag="pre"
                )
                for nn in range(nq):
                    n = qlo + nn
                    nc.vector.tensor_scalar_add(
                        pre[:, bass.ts(nn, NK)],
                        hk16[:, bass.ts(j, NK)],
                        hq32[:, j * NQ + n : j * NQ + n + 1],
                    )
                t_ = tanhpool.tile(
                    [128, nq * NK], f16, name=f"tanh{b}_{qlo}_{j}", tag="tanh"
                )
                nc.scalar.activation(t_[:], pre[:], mybir.ActivationFunctionType.Tanh)
                th.append(t_)
                if j == 0:
                    for bb, gg, w, pss in deferred:
                        ob = obpool.tile(
                            [128, 512 * w], f32, name=f"ob{bb}_{gg}", tag="ob"
                        )
                        if gg == 6:
                            nc.scalar.copy(ob[:], pss[:])
                        else:
                            nc.vector.tensor_copy(ob[:], pss[:])
                        dst = out_d[bb, gg : gg + w].rearrange("g r c -> r g c")
                        srcap = ob[0:128:32, :].rearrange("p (g c) -> p g c", g=w)
                        nc.sync.dma_start(dst, srcap)
                    deferred = []

            tails = []
            ngroups = nq // 8
            g = 0
            while g < ngroups:
                w = 2 if ngroups - g >= 2 else 1  # banks per psum tile
                ps = psO.tile(
                    [128, 512 * w], f32, name=f"ps{b}_{qlo}_{g}", tag="psO"
                )
                for gg in range(w):
                    for j in range(2):
                        for r in range(4):
                            p = 4 * (g + gg) + r
                            nc.tensor.matmul(
                                ps[32 * r : 32 * r + 32, bass.ts(gg, 512)],
                                vh_sb[:, bass.ts(j, 32)],
                                th[j][:, bass.ts(p, 512)],
                                start=(j == 0),
                                stop=(j == 1),
                                tile_position=(0, 32 * r),
                                skip_group_check=True,
                            )
                tails.append((b, qlo // 8 + g, w, ps))
                g += w

            for bb, gg, w, pss in deferred:
                ob = obpool.tile(
                    [128, 512 * w], f32, name=f"ob{bb}_{gg}", tag="ob"
                )
                if gg == 6:
                    nc.scalar.copy(ob[:], pss[:])
                else:
                    nc.vector.tensor_copy(ob[:], pss[:])
                dst = out_d[bb, gg : gg + w].rearrange("g r c -> r g c")
                srcap = ob[0:128:32, :].rearrange("p (g c) -> p g c", g=w)
                nc.sync.dma_start(dst, srcap)
            deferred = tails

        for bb, gg, w, pss in deferred:
            ob = obpool.tile([128, 512 * w], f32, name=f"ob{bb}_{gg}", tag="ob")
            nc.vector.tensor_copy(ob[:], pss[:])
            dst = out_d[bb, gg : gg + w].rearrange("g r c -> r g c")
            srcap = ob[0:128:32, :].rearrange("p (g c) -> p g c", g=w)
            nc.sync.dma_start(dst, srcap)

    nc.compile()
    return nc


def get_nc():
    if "nc" not in _NC_CACHE:
        _NC_CACHE["nc"] = build_nc()
    return _NC_CACHE["nc"]


def make_in_maps(att_query, att_key, v, W):
    att_query = np.ascontiguousarray(np.asarray(att_query, dtype=np.float32))
    att_key = np.ascontiguousarray(np.asarray(att_key, dtype=np.float32))
    v = np.asarray(v, dtype=np.float32)
    W = np.asarray(W, dtype=np.float32)

    q_all = att_query.reshape(NCORES, BPC, 2, 128, NQ)
    k_all = att_key.reshape(NCORES, BPC, 2, 128, NK)
    wqt = np.ascontiguousarray(W[:, :QH].T).reshape(2, 128, HID)
    wkt = np.ascontiguousarray(W[:, QH:].T).reshape(2, 128, HID)
    vh = np.ascontiguousarray(np.repeat(v.astype(np.float16).reshape(2, 128).T, 32, axis=1))

    return [
        {
            "q": np.ascontiguousarray(q_all[c]),
            "k": np.ascontiguousarray(k_all[c]),
            "wqt": wqt,
            "wkt": wkt,
            "vh": vh,
        }
        for c in range(NCORES)
    ]


def _ensure_ntff_hook():
    """Register the axon NTFF profile hook (image's antenv lacks axon_hooks)."""
    import types

    try:
        import antenv.axon_hooks  # noqa: F401
    except ImportError:
        import antenv

        mod = types.ModuleType("antenv.axon_hooks")
        _hook = [None]
        mod.set_axon_ntff_profile_hook = lambda h: _hook.__setitem__(0, h)
        mod.get_axon_ntff_profile_hook = lambda: _hook[0]
        sys.modules["antenv.axon_hooks"] = mod
        antenv.axon_hooks = mod
    from antenv.axon_hooks import (
        get_axon_ntff_profile_hook,
        set_axon_ntff_profile_hook,
    )

    if get_axon_ntff_profile_hook() is None:
        from trn_agent_boot.trn_boot import _ntff_profile_via_ctypes

        set_axon_ntff_profile_hook(_ntff_profile_via_ctypes("/opt/axon/libaxon_pjrt.so"))


def run(att_query, att_key, v, W, trace=False, **kwargs):
    nc = get_nc()
    if trace:
        _ensure_ntff_hook()
    in_maps = make_in_maps(att_query, att_key, v, W)
    res = run_bass_kernel_spmd(
        nc, in_maps, core_ids=list(range(NCORES)), trace=trace, **kwargs
    )
    outs = [np.asarray(res.results[c]["out"]).reshape(BPC, NQ * NK) for c in range(NCORES)]
    return np.concatenate(outs, axis=0), res


def kernel(att_query, att_key, v, W):
    out, _ = run(att_query, att_key, v, W)
    return out


# revision 22
# speedup vs baseline: 1.0116x; 1.0021x over previous
"""Additive-attention (Bahdanau) kernel for Trainium2, 8 NeuronCores.

Computes attns[b, n, m] = sum_h v[h] * tanh(hq[b, h, n] + hk[b, h, m])
where hq = Wq @ q[b], hk = Wk @ k[b], returned flattened as (B, NQ*NK).

Strategy (data-parallel over batch, 4 batches per core):
  - hq/hk via fp32 PE matmuls (host-pretransposed W as lhsT), cast to fp16.
  - preact[h, (n,m)] = hk + hq[:, n] built per-query with DVE
    tensor_scalar_add (per-partition scalar operand -> 4x mode on fp16).
  - tanh on ScalarE in big (128, 8192) instructions (the bottleneck engine:
    ~16.8M tanh elems/core at 128/cycle @ 1.2 GHz ~= 110us).
  - v-contraction over h on PE: v_half (128,1) stationary, tanh slab rhs
    N=512 per matmul, 2 h-halves accumulated in PSUM; 4 query-pairs share
    one PSUM bank via col-tiling (tile_position) at partitions 0/32/64/96.
  - PSUM->SBUF on DVE, one strided DMA per 4-pair group to HBM.
"""

import sys

sys.path.insert(0, "/opt/trn_rl_repo")

from contextlib import ExitStack

import numpy as np

import concourse.bacc as bacc
import concourse.bass as bass
import concourse.mybir as mybir
import concourse.tile as tile
from concourse.bass_utils import run_bass_kernel_spmd

B, HID, QH, KH, NQ, NK = 32, 256, 256, 256, 64, 256
NCORES = 8
BPC = B // NCORES  # batches per core
NCHUNK = 2  # query chunks per batch
QPC = NQ // NCHUNK  # queries per chunk (32)
PAIRS = QPC // 2  # query pairs per chunk (16)
GROUPS = PAIRS // 4  # groups of 4 pairs per chunk (4)

f32 = mybir.dt.float32
f16 = mybir.dt.float16

_NC_CACHE = {}


def build_nc():
    nc = bacc.Bacc("TRN2", target_bir_lowering=False, debug=False)

    q_d = nc.dram_tensor("q", [BPC, 2, 128, NQ], f32, kind="ExternalInput")
    k_d = nc.dram_tensor("k", [BPC, 2, 128, NK], f32, kind="ExternalInput")
    wqt_d = nc.dram_tensor("wqt", [2, 128, HID], f32, kind="ExternalInput")
    wkt_d = nc.dram_tensor("wkt", [2, 128, HID], f32, kind="ExternalInput")
    vh_d = nc.dram_tensor("vh", [128, 64], f16, kind="ExternalInput")
    out_d = nc.dram_tensor("out", [BPC, 2 * GROUPS, 4, 512], f32, kind="ExternalOutput")

    with tile.TileContext(nc) as tc, ExitStack() as ctx:
        wpool = ctx.enter_context(tc.tile_pool(name="wpool", bufs=1))
        iopool = ctx.enter_context(tc.tile_pool(name="iopool", bufs=3))
        hpool = ctx.enter_context(tc.tile_pool(name="hpool", bufs=3))
        prepool = ctx.enter_context(tc.tile_pool(name="prepool", bufs=3))
        tanhpool = ctx.enter_context(tc.tile_pool(name="tanhpool", bufs=4))
        obpool = ctx.enter_context(tc.tile_pool(name="obpool", bufs=6))
        psA = ctx.enter_context(tc.tile_pool(name="psA", bufs=2, space="PSUM"))
        psO = ctx.enter_context(tc.tile_pool(name="psO", bufs=3, space="PSUM"))

        # Preload the tanh ACT table at t=0 (overlaps with input DMAs).
        warm = wpool.tile([128, 2], f16, name="warm", tag="warm")
        nc.vector.memset(warm[:, 0:1], 0.0)
        nc.scalar.activation(
            warm[:, 1:2], warm[:, 0:1], mybir.ActivationFunctionType.Tanh
        )

        def load_qk(b, eng=None):
            eng = eng or nc.gpsimd
            q_sb = iopool.tile([128, 2 * NQ], f32, name=f"q_sb{b}", tag="qsb")
            k_sb = iopool.tile([128, 2 * NK], f32, name=f"k_sb{b}", tag="ksb")
            eng.dma_start(
                q_sb[:].rearrange("p (kb n) -> p kb n", kb=2),
                q_d[b].rearrange("kb p n -> p kb n"),
            )
            eng.dma_start(
                k_sb[:].rearrange("p (kb n) -> p kb n", kb=2),
                k_d[b].rearrange("kb p n -> p kb n"),
            )
            return q_sb, k_sb

        q0_sb = iopool.tile([128, 2 * NQ], f32, name="q_sb0", tag="qsb")
        k0_sb = iopool.tile([128, 2 * NK], f32, name="k_sb0", tag="ksb")
        wq_sb = []
        wk_sb = []
        for kb in range(2):
            wq_t = wpool.tile([128, HID], f32, name=f"wq_sb{kb}", tag=f"wq{kb}")
            wq_sb.append(wq_t)
            wk_t = wpool.tile([128, HID], f32, name=f"wk_sb{kb}", tag=f"wk{kb}")
            wk_sb.append(wk_t)
        vh_sb = wpool.tile([128, 64], f16, name="vh_sb", tag="vh")
        # Split critical startup DMA issue across the two HWDGE engines:
        # sync takes the hq inputs, scalar takes the hk inputs. One DMA per
        # tensor (3D AP over the kb halves) to minimize serialized issues.
        nc.sync.dma_start(
            q0_sb[:].rearrange("p (kb n) -> p kb n", kb=2),
            q_d[0].rearrange("kb p n -> p kb n"),
        )
        nc.sync.dma_start(wq_sb[0][:], wqt_d[0])
        nc.sync.dma_start(wq_sb[1][:], wqt_d[1])
        nc.scalar.dma_start(
            k0_sb[:].rearrange("p (kb n) -> p kb n", kb=2),
            k_d[0].rearrange("kb p n -> p kb n"),
        )
        nc.scalar.dma_start(wk_sb[0][:], wkt_d[0])
        nc.scalar.dma_start(wk_sb[1][:], wkt_d[1])
        nc.scalar.dma_start(vh_sb[:], vh_d[:])
        qk = {0: (q0_sb, k0_sb)}
        hqhk = {}
        deferred = []
        done_tiles = []

        def make_hqhk(b):
            cast = nc.scalar.copy if b == 0 else nc.vector.tensor_copy
            q_sb, k_sb = qk.pop(b)
            hq32 = hpool.tile([128, 2 * NQ], f32, name=f"hq32_{b}", tag="hq32")
            hk16 = hpool.tile([128, 2 * NK], f16, name=f"hk16_{b}", tag="hk16")
            for j in range(2):
                ps_hq = psA.tile([128, NQ], f32, name=f"ps_hq{b}_{j}", tag="psA")
                for kb in range(2):
                    nc.tensor.matmul(
                        ps_hq[:],
                        wq_sb[kb][:, bass.ts(j, 128)],
                        q_sb[:, bass.ts(kb, NQ)],
                        start=(kb == 0),
                        stop=(kb == 1),
                    )
                cast(hq32[:, bass.ts(j, NQ)], ps_hq[:])
                ps_hk = psA.tile([128, NK], f32, name=f"ps_hk{b}_{j}", tag="psA")
                for kb in range(2):
                    nc.tensor.matmul(
                        ps_hk[:],
                        wk_sb[kb][:, bass.ts(j, 128)],
                        k_sb[:, bass.ts(kb, NK)],
                        start=(kb == 0),
                        stop=(kb == 1),
                    )
                cast(hk16[:, bass.ts(j, NK)], ps_hk[:])
            hqhk[b] = (hq32, hk16)

        make_hqhk(0)
        qk[1] = load_qk(1)
        make_hqhk(1)
        qk[2] = load_qk(2)

        # Work units: (batch, qlo, nq). Fine-grained at the start so ACT
        # ramps early, 16-query pieces at the end for a short drain; full
        # 32-query chunks in steady state.
        units = []
        for b in range(BPC):
            if b == 0:
                units += [(0, 0, 8), (0, 8, 8), (0, 16, 16), (0, 32, 32)]
            elif b == BPC - 1:
                units += [(b, 0, 32), (b, 32, 16), (b, 48, 8), (b, 56, 8)]
            else:
                units += [(b, 0, 32), (b, 32, 32)]

        deferred = []
        seen_chunks = 0
        for ui, (b, qlo, nq) in enumerate(units):
            hq32, hk16 = hqhk[b]
            if qlo == 0:
                if b + 2 < BPC:
                    make_hqhk(b + 2)
                if b + 3 < BPC:
                    qk[b + 3] = load_qk(b + 3)

            th = []
            for j in range(2):
                pre = prepool.tile(
                    [128, nq * NK], f16, name=f"pre{b}_{qlo}_{j}", tag="pre"
                )
                for nn in range(nq):
                    n = qlo + nn
                    nc.vector.tensor_scalar_add(
                        pre[:, bass.ts(nn, NK)],
                        hk16[:, bass.ts(j, NK)],
                        hq32[:, j * NQ + n : j * NQ + n + 1],
                    )
                t_ = tanhpool.tile(
                    [128, nq * NK], f16, name=f"tanh{b}_{qlo}_{j}", tag="tanh"
                )
                nc.scalar.activation(t_[:], pre[:], mybir.ActivationFunctionType.Tanh)
                th.append(t_)
                if j == 0:
                    for bb, gg, w, pss in deferred:
                        ob = obpool.tile(
                            [128, 512 * w], f32, name=f"ob{bb}_{gg}", tag="ob"
                        )
                        nc.vector.tensor_copy(ob[:], pss[:])
                        dst = out_d[bb, gg : gg + w].rearrange("g r c -> r g c")
                        srcap = ob[0:128:32, :].rearrange("p (g c) -> p g c", g=w)
                        nc.sync.dma_start(dst, srcap)
                    deferred = []

            tails = []
            ngroups = nq // 8
            g = 0
            while g < ngroups:
                w = 2 if ngroups - g >= 2 else 1  # banks per psum tile
                ps = psO.tile(
                    [128, 512 * w], f32, name=f"ps{b}_{qlo}_{g}", tag="psO"
                )
                for gg in range(w):
                    for j in range(2):
                        for r in range(4):
                            p = 4 * (g + gg) + r
                            nc.tensor.matmul(
                                ps[32 * r : 32 * r + 32, bass.ts(gg, 512)],
                                vh_sb[:, bass.ts(j, 32)],
                                th[j][:, bass.ts(p, 512)],
                                start=(j == 0),
                                stop=(j == 1),
                                tile_position=(0, 32 * r),
                                skip_group_check=True,
                            )
                tails.append((b, qlo // 8 + g, w, ps))
                g += w

            for bb, gg, w, pss in deferred:
                ob = obpool.tile(
                    [128, 512 * w], f32, name=f"ob{bb}_{gg}", tag="ob"
                )
                if gg == 6:
                    nc.scalar.copy(ob[:], pss[:])
                else:
                    nc.vector.tensor_copy(ob[:], pss[:])
                dst = out_d[bb, gg : gg + w].rearrange("g r c -> r g c")
                srcap = ob[0:128:32, :].rearrange("p (g c) -> p g c", g=w)
                nc.sync.dma_start(dst, srcap)
            deferred = tails

        for bb, gg, w, pss in deferred:
            ob = obpool.tile([128, 512 * w], f32, name=f"ob{bb}_{gg}", tag="ob")
            nc.vector.tensor_copy(ob[:], pss[:])
            dst = out_d[bb, gg : gg + w].rearrange("g r c -> r g c")
            srcap = ob[0:128:32, :].rearrange("p (g c) -> p g c", g=w)
            nc.sync.dma_start(dst, srcap)

    nc.compile()
    return nc


def get_nc():
    if "nc" not in _NC_CACHE:
        _NC_CACHE["nc"] = build_nc()
    return _NC_CACHE["nc"]


def make_in_maps(att_query, att_key, v, W):
    att_query = np.ascontiguousarray(np.asarray(att_query, dtype=np.float32))
    att_key = np.ascontiguousarray(np.asarray(att_key, dtype=np.float32))
    v = np.asarray(v, dtype=np.float32)
    W = np.asarray(W, dtype=np.float32)

    q_all = att_query.reshape(NCORES, BPC, 2, 128, NQ)
    k_all = att_key.reshape(NCORES, BPC, 2, 128, NK)
    wqt = np.ascontiguousarray(W[:, :QH].T).reshape(2, 128, HID)
    wkt = np.ascontiguousarray(W[:, QH:].T).reshape(2, 128, HID)
    vh = np.ascontiguousarray(np.repeat(v.astype(np.float16).reshape(2, 128).T, 32, axis=1))

    return [
        {
            "q": np.ascontiguousarray(q_all[c]),
            "k": np.ascontiguousarray(k_all[c]),
            "wqt": wqt,
            "wkt": wkt,
            "vh": vh,
        }
        for c in range(NCORES)
    ]


def _ensure_ntff_hook():
    """Register the axon NTFF profile hook (image's antenv lacks axon_hooks)."""
    import types

    try:
        import antenv.axon_hooks  # noqa: F401
    except ImportError:
        import antenv

        mod = types.ModuleType("antenv.axon_hooks")
        _hook = [None]
        mod.set_axon_ntff_profile_hook = lambda h: _hook.__setitem__(0, h)
        mod.get_axon_ntff_profile_hook = lambda: _hook[0]
        sys.modules["antenv.axon_hooks"] = mod
        antenv.axon_hooks = mod
    from antenv.axon_hooks import (
        get_axon_ntff_profile_hook,
        set_axon_ntff_profile_hook,
    )

    if get_axon_ntff_profile_hook() is None:
        from trn_agent_boot.trn_boot import _ntff_profile_via_ctypes

        set_axon_ntff_profile_hook(_ntff_profile_via_ctypes("/opt/axon/libaxon_pjrt.so"))


def run(att_query, att_key, v, W, trace=False, **kwargs):
    nc = get_nc()
    if trace:
        _ensure_ntff_hook()
    in_maps = make_in_maps(att_query, att_key, v, W)
    res = run_bass_kernel_spmd(
        nc, in_maps, core_ids=list(range(NCORES)), trace=trace, **kwargs
    )
    outs = [np.asarray(res.results[c]["out"]).reshape(BPC, NQ * NK) for c in range(NCORES)]
    return np.concatenate(outs, axis=0), res


def kernel(att_query, att_key, v, W):
    out, _ = run(att_query, att_key, v, W)
    return out


# revision 23
# speedup vs baseline: 1.0158x; 1.0041x over previous
"""Additive-attention (Bahdanau) kernel for Trainium2, 8 NeuronCores.

Computes attns[b, n, m] = sum_h v[h] * tanh(hq[b, h, n] + hk[b, h, m])
where hq = Wq @ q[b], hk = Wk @ k[b], returned flattened as (B, NQ*NK).

Strategy (data-parallel over batch, 4 batches per core):
  - hq/hk via fp32 PE matmuls (host-pretransposed W as lhsT); hq kept
    fp32 (scalar operand), hk cast fp16.
  - preact[h, (n,m)] = hk + hq[:, n] built per-query with DVE
    tensor_scalar_add (fp16 streams at 2x mode, ~196ns per 128x256).
  - tanh on ScalarE in big fp16 instructions -- the bottleneck engine:
    ~16.8M tanh elems/core at 128 lanes @ 1.2 GHz ~= 114us busy.
  - v-contraction over h on PE: v half replicated to (128,32) stationary,
    fp16 tanh slab rhs N=512 per matmul, 2 h-halves accumulated in PSUM;
    4 query-pairs share each PSUM bank via col-tiling (tile_position) at
    partitions 0/32/64/96; two banks per PSUM tile.
  - PSUM->SBUF copy on DVE (deferred one unit to keep DVE streaming),
    strided DMA to HBM. Both DVE and ACT end ~120us busy; ~151us wall.
"""

import sys

sys.path.insert(0, "/opt/trn_rl_repo")

from contextlib import ExitStack

import numpy as np

import concourse.bacc as bacc
import concourse.bass as bass
import concourse.mybir as mybir
import concourse.tile as tile
from concourse.bass_utils import run_bass_kernel_spmd

B, HID, QH, KH, NQ, NK = 32, 256, 256, 256, 64, 256
NCORES = 8
BPC = B // NCORES  # batches per core
NCHUNK = 2  # query chunks per batch
QPC = NQ // NCHUNK  # queries per chunk (32)
PAIRS = QPC // 2  # query pairs per chunk (16)
GROUPS = PAIRS // 4  # groups of 4 pairs per chunk (4)

f32 = mybir.dt.float32
f16 = mybir.dt.float16

_NC_CACHE = {}


def build_nc():
    nc = bacc.Bacc("TRN2", target_bir_lowering=False, debug=False)

    q_d = nc.dram_tensor("q", [BPC, 2, 128, NQ], f32, kind="ExternalInput")
    k_d = nc.dram_tensor("k", [BPC, 2, 128, NK], f32, kind="ExternalInput")
    wqt_d = nc.dram_tensor("wqt", [2, 128, HID], f32, kind="ExternalInput")
    wkt_d = nc.dram_tensor("wkt", [2, 128, HID], f32, kind="ExternalInput")
    vh_d = nc.dram_tensor("vh", [128, 64], f16, kind="ExternalInput")
    out_d = nc.dram_tensor("out", [BPC, 2 * GROUPS, 4, 512], f32, kind="ExternalOutput")

    with tile.TileContext(nc) as tc, ExitStack() as ctx:
        wpool = ctx.enter_context(tc.tile_pool(name="wpool", bufs=1))
        iopool = ctx.enter_context(tc.tile_pool(name="iopool", bufs=3))
        hpool = ctx.enter_context(tc.tile_pool(name="hpool", bufs=3))
        prepool = ctx.enter_context(tc.tile_pool(name="prepool", bufs=3))
        tanhpool = ctx.enter_context(tc.tile_pool(name="tanhpool", bufs=4))
        obpool = ctx.enter_context(tc.tile_pool(name="obpool", bufs=6))
        psA = ctx.enter_context(tc.tile_pool(name="psA", bufs=2, space="PSUM"))
        psO = ctx.enter_context(tc.tile_pool(name="psO", bufs=3, space="PSUM"))

        # Preload the tanh ACT table at t=0 (overlaps with input DMAs).
        warm = wpool.tile([128, 2], f16, name="warm", tag="warm")
        nc.vector.memset(warm[:, 0:1], 0.0)
        nc.scalar.activation(
            warm[:, 1:2], warm[:, 0:1], mybir.ActivationFunctionType.Tanh
        )

        def load_qk(b, eng=None):
            eng = eng or nc.gpsimd
            q_sb = iopool.tile([128, 2 * NQ], f32, name=f"q_sb{b}", tag="qsb")
            k_sb = iopool.tile([128, 2 * NK], f32, name=f"k_sb{b}", tag="ksb")
            eng.dma_start(
                q_sb[:].rearrange("p (kb n) -> p kb n", kb=2),
                q_d[b].rearrange("kb p n -> p kb n"),
            )
            eng.dma_start(
                k_sb[:].rearrange("p (kb n) -> p kb n", kb=2),
                k_d[b].rearrange("kb p n -> p kb n"),
            )
            return q_sb, k_sb

        q0_sb = iopool.tile([128, 2 * NQ], f32, name="q_sb0", tag="qsb")
        k0_sb = iopool.tile([128, 2 * NK], f32, name="k_sb0", tag="ksb")
        wq_sb = []
        wk_sb = []
        for kb in range(2):
            wq_t = wpool.tile([128, HID], f32, name=f"wq_sb{kb}", tag=f"wq{kb}")
            wq_sb.append(wq_t)
            wk_t = wpool.tile([128, HID], f32, name=f"wk_sb{kb}", tag=f"wk{kb}")
            wk_sb.append(wk_t)
        vh_sb = wpool.tile([128, 64], f16, name="vh_sb", tag="vh")
        # Split critical startup DMA issue across the two HWDGE engines:
        # sync takes the hq inputs, scalar takes the hk inputs. One DMA per
        # tensor (3D AP over the kb halves) to minimize serialized issues.
        nc.sync.dma_start(
            q0_sb[:].rearrange("p (kb n) -> p kb n", kb=2),
            q_d[0].rearrange("kb p n -> p kb n"),
        )
        nc.sync.dma_start(wq_sb[0][:], wqt_d[0])
        nc.sync.dma_start(wq_sb[1][:], wqt_d[1])
        nc.scalar.dma_start(
            k0_sb[:].rearrange("p (kb n) -> p kb n", kb=2),
            k_d[0].rearrange("kb p n -> p kb n"),
        )
        nc.scalar.dma_start(wk_sb[0][:], wkt_d[0])
        nc.scalar.dma_start(wk_sb[1][:], wkt_d[1])
        nc.scalar.dma_start(vh_sb[:], vh_d[:])
        qk = {0: (q0_sb, k0_sb)}
        hqhk = {}

        def make_hqhk(b):
            cast = nc.scalar.copy if b == 0 else nc.vector.tensor_copy
            q_sb, k_sb = qk.pop(b)
            hq32 = hpool.tile([128, 2 * NQ], f32, name=f"hq32_{b}", tag="hq32")
            hk16 = hpool.tile([128, 2 * NK], f16, name=f"hk16_{b}", tag="hk16")
            for j in range(2):
                ps_hq = psA.tile([128, NQ], f32, name=f"ps_hq{b}_{j}", tag="psA")
                for kb in range(2):
                    nc.tensor.matmul(
                        ps_hq[:],
                        wq_sb[kb][:, bass.ts(j, 128)],
                        q_sb[:, bass.ts(kb, NQ)],
                        start=(kb == 0),
                        stop=(kb == 1),
                    )
                cast(hq32[:, bass.ts(j, NQ)], ps_hq[:])
                ps_hk = psA.tile([128, NK], f32, name=f"ps_hk{b}_{j}", tag="psA")
                for kb in range(2):
                    nc.tensor.matmul(
                        ps_hk[:],
                        wk_sb[kb][:, bass.ts(j, 128)],
                        k_sb[:, bass.ts(kb, NK)],
                        start=(kb == 0),
                        stop=(kb == 1),
                    )
                cast(hk16[:, bass.ts(j, NK)], ps_hk[:])
            hqhk[b] = (hq32, hk16)

        make_hqhk(0)
        qk[1] = load_qk(1)
        make_hqhk(1)
        qk[2] = load_qk(2)

        # Work units: (batch, qlo, nq). Fine-grained at the start so ACT
        # ramps early, 16-query pieces at the end for a short drain; full
        # 32-query chunks in steady state.
        units = []
        for b in range(BPC):
            if b == 0:
                units += [(0, 0, 8), (0, 8, 8), (0, 16, 16), (0, 32, 32)]
            elif b == BPC - 1:
                units += [(b, 0, 32), (b, 32, 16), (b, 48, 8), (b, 56, 8)]
            else:
                units += [(b, 0, 32), (b, 32, 32)]

        deferred = []
        for b, qlo, nq in units:
            hq32, hk16 = hqhk[b]
            if qlo == 0:
                if b + 2 < BPC:
                    make_hqhk(b + 2)
                if b + 3 < BPC:
                    qk[b + 3] = load_qk(b + 3)

            th = []
            for j in range(2):
                pre = prepool.tile(
                    [128, nq * NK], f16, name=f"pre{b}_{qlo}_{j}", tag="pre"
                )
                for nn in range(nq):
                    n = qlo + nn
                    nc.vector.tensor_scalar_add(
                        pre[:, bass.ts(nn, NK)],
                        hk16[:, bass.ts(j, NK)],
                        hq32[:, j * NQ + n : j * NQ + n + 1],
                    )
                t_ = tanhpool.tile(
                    [128, nq * NK], f16, name=f"tanh{b}_{qlo}_{j}", tag="tanh"
                )
                nc.scalar.activation(t_[:], pre[:], mybir.ActivationFunctionType.Tanh)
                th.append(t_)
                if j == 0:
                    for bb, gg, w, pss in deferred:
                        ob = obpool.tile(
                            [128, 512 * w], f32, name=f"ob{bb}_{gg}", tag="ob"
                        )
                        nc.vector.tensor_copy(ob[:], pss[:])
                        dst = out_d[bb, gg : gg + w].rearrange("g r c -> r g c")
                        srcap = ob[0:128:32, :].rearrange("p (g c) -> p g c", g=w)
                        nc.sync.dma_start(dst, srcap)
                    deferred = []

            tails = []
            ngroups = nq // 8
            g = 0
            while g < ngroups:
                w = 2 if ngroups - g >= 2 else 1  # banks per psum tile
                ps = psO.tile(
                    [128, 512 * w], f32, name=f"ps{b}_{qlo}_{g}", tag="psO"
                )
                for gg in range(w):
                    for j in range(2):
                        for r in range(4):
                            p = 4 * (g + gg) + r
                            nc.tensor.matmul(
                                ps[32 * r : 32 * r + 32, bass.ts(gg, 512)],
                                vh_sb[:, bass.ts(j, 32)],
                                th[j][:, bass.ts(p, 512)],
                                start=(j == 0),
                                stop=(j == 1),
                                tile_position=(0, 32 * r),
                                skip_group_check=True,
                            )
                tails.append((b, qlo // 8 + g, w, ps))
                g += w

            deferred = tails

        for bb, gg, w, pss in deferred:
            ob = obpool.tile([128, 512 * w], f32, name=f"ob{bb}_{gg}", tag="ob")
            nc.vector.tensor_copy(ob[:], pss[:])
            dst = out_d[bb, gg : gg + w].rearrange("g r c -> r g c")
            srcap = ob[0:128:32, :].rearrange("p (g c) -> p g c", g=w)
            nc.sync.dma_start(dst, srcap)

    nc.compile()
    return nc


def get_nc():
    if "nc" not in _NC_CACHE:
        _NC_CACHE["nc"] = build_nc()
    return _NC_CACHE["nc"]


def make_in_maps(att_query, att_key, v, W):
    att_query = np.ascontiguousarray(np.asarray(att_query, dtype=np.float32))
    att_key = np.ascontiguousarray(np.asarray(att_key, dtype=np.float32))
    v = np.asarray(v, dtype=np.float32)
    W = np.asarray(W, dtype=np.float32)

    q_all = att_query.reshape(NCORES, BPC, 2, 128, NQ)
    k_all = att_key.reshape(NCORES, BPC, 2, 128, NK)
    wqt = np.ascontiguousarray(W[:, :QH].T).reshape(2, 128, HID)
    wkt = np.ascontiguousarray(W[:, QH:].T).reshape(2, 128, HID)
    vh = np.ascontiguousarray(np.repeat(v.astype(np.float16).reshape(2, 128).T, 32, axis=1))

    return [
        {
            "q": np.ascontiguousarray(q_all[c]),
            "k": np.ascontiguousarray(k_all[c]),
            "wqt": wqt,
            "wkt": wkt,
            "vh": vh,
        }
        for c in range(NCORES)
    ]


def _ensure_ntff_hook():
    """Register the axon NTFF profile hook (image's antenv lacks axon_hooks)."""
    import types

    try:
        import antenv.axon_hooks  # noqa: F401
    except ImportError:
        import antenv

        mod = types.ModuleType("antenv.axon_hooks")
        _hook = [None]
        mod.set_axon_ntff_profile_hook = lambda h: _hook.__setitem__(0, h)
        mod.get_axon_ntff_profile_hook = lambda: _hook[0]
        sys.modules["antenv.axon_hooks"] = mod
        antenv.axon_hooks = mod
    from antenv.axon_hooks import (
        get_axon_ntff_profile_hook,
        set_axon_ntff_profile_hook,
    )

    if get_axon_ntff_profile_hook() is None:
        from trn_agent_boot.trn_boot import _ntff_profile_via_ctypes

        set_axon_ntff_profile_hook(_ntff_profile_via_ctypes("/opt/axon/libaxon_pjrt.so"))


def run(att_query, att_key, v, W, trace=False, **kwargs):
    nc = get_nc()
    if trace:
        _ensure_ntff_hook()
    in_maps = make_in_maps(att_query, att_key, v, W)
    res = run_bass_kernel_spmd(
        nc, in_maps, core_ids=list(range(NCORES)), trace=trace, **kwargs
    )
    outs = [np.asarray(res.results[c]["out"]).reshape(BPC, NQ * NK) for c in range(NCORES)]
    return np.concatenate(outs, axis=0), res


def kernel(att_query, att_key, v, W):
    out, _ = run(att_query, att_key, v, W)
    return out
